# revision 1
# baseline (speedup 1.0000x reference)
"""Banded causal self-attention (B=1, T=4096, C=1024, H=16, Dh=64, band=128)
on 8 Trainium2 NeuronCores, sequence-parallel (512 queries/core + 128-row halo).

Layout strategy (host pre-transposes, so zero on-chip weight transposes):
  - feeds x^T slice (C, 640) per core; Wq^T (scaled by 1/sqrt(Dh)), Wk^T,
    Wv^T, Wo^T as (C, C) contraction-major arrays.
  - q^T/k^T computed as (o, t) tiles; v as (t, o); attention scores banded
    (each 128-query block sees exactly 2 key blocks); softmax along free dim
    without max-subtraction (scores are O(1) by construction); P transposed
    via PE; y^T accumulated per head; output projection back to (t, u).
"""

import os
import sys

import ml_dtypes
import numpy as np

sys.path.insert(0, "/opt/trn_rl_repo")

import concourse.bass as bass  # noqa: F401
import concourse.mybir as mybir
import concourse.tile as tile
from concourse import bacc
from concourse.bass_utils import run_bass_kernel_spmd
from concourse.masks import make_identity

T, C, H, DH = 4096, 1024, 16, 64
BAND = 128
NCORES = 8
TLOC = T // NCORES          # 512 queries per core
HALO = BAND                 # 128
KV = TLOC + HALO            # 640 kv rows per core
NQB = TLOC // 128           # 4 query blocks
NKB = KV // 128             # 5 kv blocks
KT = C // 128               # 8 contraction tiles
F32 = mybir.dt.float32
F32R = mybir.dt.float32r
BF16 = mybir.dt.bfloat16
MULT = mybir.AluOpType.mult
ADD = mybir.AluOpType.add
EXP = mybir.ActivationFunctionType.Exp

_cached = {}


def build_nc():
    nc = bacc.Bacc(
        "TRN2",
        target_bir_lowering=False,
        debug=False,
        num_devices=NCORES,
    )

    xt_d = nc.dram_tensor("xt", [C, KV], BF16, kind="ExternalInput")
    wqt_d = nc.dram_tensor("wqt", [C, C], BF16, kind="ExternalInput")
    wkt_d = nc.dram_tensor("wkt", [C, C], BF16, kind="ExternalInput")
    wvt_d = nc.dram_tensor("wvt", [C, C], BF16, kind="ExternalInput")
    wot_d = nc.dram_tensor("wot", [C, C], BF16, kind="ExternalInput")
    m0_d = nc.dram_tensor("mask0", [128, 256], F32, kind="ExternalInput")
    mr_d = nc.dram_tensor("maskr", [128, 256], F32, kind="ExternalInput")
    out_d = nc.dram_tensor("out", [TLOC, C], F32, kind="ExternalOutput")

    with tile.TileContext(nc) as tc:
        with (
            tc.tile_pool(name="const", bufs=1) as constp,
            tc.tile_pool(name="xt", bufs=KT) as xtp,
            tc.tile_pool(name="w", bufs=16) as wp,
            tc.tile_pool(name="qt", bufs=KT) as qtp,
            tc.tile_pool(name="kt", bufs=KT) as ktp,
            tc.tile_pool(name="v", bufs=NKB) as vp,
            tc.tile_pool(name="yt", bufs=KT) as ytp,
            tc.tile_pool(name="att", bufs=6) as attp,
            tc.tile_pool(name="pt", bufs=2 * NKB) as ptp,
            tc.tile_pool(name="stat", bufs=8) as statp,
            tc.tile_pool(name="z", bufs=2) as zp,
            tc.tile_pool(name="psum", bufs=1, space="PSUM") as psp,
        ):
            # constants
            ident = constp.tile([128, 128], BF16, name="ident")
            make_identity(nc, ident[:])
            # HAM warm-up: junk matmuls that run while the first DMAs land,
            # flipping the PE clock gate to 8/8 before real work begins
            junk = constp.tile([128, 512], BF16, name="junk")
            nc.vector.memset(junk[:], 0.0)
            ps_w = psp.tile([128, 512], F32, tag="y", bufs=2, name="warm")
            for _ in range(16):
                nc.tensor.matmul(ps_w[:], junk[:, 0:128], junk[:], start=True,
                                 stop=True)

            mb01 = constp.tile([128, 512], F32, name="mb01")
            mbr2 = constp.tile([128, 512], F32, name="mbr2")
            nc.sync.dma_start(mb01[:, 0:256], m0_d[:])
            nc.sync.dma_start(mb01[:, 256:512], mr_d[:])
            nc.sync.dma_start(mbr2[:, 0:256], mr_d[:])
            nc.sync.dma_start(mbr2[:, 256:512], mr_d[:])

            def load_w(dram, base, k):
                w = wp.tile([128, C], BF16, name=f"{base}{k}", tag="w", bufs=16)
                nc.sync.dma_start(w[:], dram[k * 128:(k + 1) * 128, :])
                return w

            # interleave x^T and Wq tile loads so the first q-projection
            # accumulation chain starts as early as possible
            xt_t, wq_t = [], []
            for a in range(KT):
                xt = xtp.tile([128, KV], BF16, name=f"xt{a}", tag="xt", bufs=KT)
                nc.sync.dma_start(xt[:], xt_d[a * 128:(a + 1) * 128, :])
                xt_t.append(xt)
                wq_t.append(load_w(wqt_d, "wq", a))
            wk_t = [load_w(wkt_d, "wk", k) for k in range(KT)]

            # ---- q^T projection: out (o, t) tiles [128, 512]
            qt_t = []
            for o in range(KT):
                ps = psp.tile([128, 512], F32, tag="proj", bufs=3, name=f"psq{o}")
                for k in range(KT):
                    nc.tensor.matmul(
                        ps[:],
                        wq_t[k][:, o * 128:(o + 1) * 128],
                        xt_t[k][:, HALO:],
                        start=(k == 0),
                        stop=(k == KT - 1),
                    )
                qt = qtp.tile([128, TLOC], F32R, name=f"qt{o}", tag="qt", bufs=KT)
                nc.scalar.copy(qt[:], ps[:])
                qt_t.append(qt)

            # ---- k^T projection: out (o, t) tiles [128, 640]
            kt_t = []
            for o in range(KT):
                kt = ktp.tile([128, KV], F32R, name=f"kt{o}", tag="kt", bufs=KT)
                for n0, nw in ((0, 384), (384, 256)):
                    ps = psp.tile([128, 512], F32, tag="proj", bufs=3, name=f"psk{o}_{n0}")
                    for k in range(KT):
                        nc.tensor.matmul(
                            ps[:, :nw],
                            wk_t[k][:, o * 128:(o + 1) * 128],
                            xt_t[k][:, n0:n0 + nw],
                            start=(k == 0),
                            stop=(k == KT - 1),
                        )
                    nc.scalar.copy(kt[:, n0:n0 + nw], ps[:, :nw])
                kt_t.append(kt)

            wv_t = [load_w(wvt_d, "wv", k) for k in range(KT)]

            # ---- v projection: out (t, o) tiles [128, 1024]
            v_t = []
            for tb in range(NKB):
                v = vp.tile([128, C], BF16, name=f"v{tb}", tag="v", bufs=NKB)
                for n0 in (0, 512):
                    ps = psp.tile([128, 512], F32, tag="proj", bufs=3, name=f"psv{tb}_{n0}")
                    for k in range(KT):
                        nc.tensor.matmul(
                            ps[:],
                            xt_t[k][:, tb * 128:(tb + 1) * 128],
                            wv_t[k][:, n0:n0 + 512],
                            start=(k == 0),
                            stop=(k == KT - 1),
                        )
                    nc.scalar.copy(v[:, n0:n0 + 512], ps[:])
                v_t.append(v)



            wo_t = [load_w(wot_d, "wo", k) for k in range(KT)]

            # ---- banded attention: head pairs (concurrent PE row-groups)
            # x query-block pairs batched into shared PSUM banks
            yt_t = [None] * KT
            pt_all = []
            for g in range(KT):
                hs = (2 * g, 2 * g + 1)
                pt_t = {h: ptp.tile([128, 256 * NKB], BF16,
                                    name=f"pt{h}", tag="pt", bufs=16)
                        for h in hs}
                pt_all.append(pt_t)
                for qp in range(NQB // 2):
                    qbs = (2 * qp, 2 * qp + 1)
                    mb = mb01 if qp == 0 else mbr2
                    ps_s, sm, e, den, rec = {}, {}, {}, {}, {}
                    for h in hs:
                        ho = (h % 2) * 64
                        ps_s[h] = psp.tile([128, 512], F32, tag="s", bufs=3,
                                           name=f"s{h}_{qp}")
                        for i, qb in enumerate(qbs):
                            nc.tensor.matmul(
                                ps_s[h][:, i * 256:(i + 1) * 256],
                                qt_t[g][ho:ho + 64, qb * 128:(qb + 1) * 128],
                                kt_t[g][ho:ho + 64, qb * 128:qb * 128 + 256],
                                start=True,
                                stop=True,
                            )
                    for h in hs:
                        sm[h] = attp.tile([128, 512], F32, tag="sm", bufs=6,
                                          name=f"sm{h}_{qp}")
                        nc.vector.tensor_add(sm[h][:], ps_s[h][:], mb[:])
                    for h in hs:
                        e[h] = attp.tile([128, 512], BF16, tag="e", bufs=6,
                                         name=f"e{h}_{qp}")
                        den[h] = statp.tile([128, 2], F32, tag="den", bufs=8,
                                            name=f"den{h}_{qp}")
                        for i in range(2):
                            nc.scalar.activation(
                                e[h][:, i * 256:(i + 1) * 256],
                                sm[h][:, i * 256:(i + 1) * 256], EXP,
                                accum_out=den[h][:, i:i + 1])
                    for h in hs:
                        rec[h] = statp.tile([128, 2], F32, tag="rec", bufs=8,
                                            name=f"rec{h}_{qp}")
                        nc.vector.reciprocal(rec[h][:], den[h][:])
                    for h in hs:
                        ps_t = psp.tile([128, 512], BF16, tag="proj", bufs=3,
                                        name=f"t{h}_{qp}")
                        for i, qb in enumerate(qbs):
                            p = attp.tile([128, 256], BF16, tag="p", bufs=6,
                                          name=f"p{h}_{qb}")
                            nc.vector.tensor_scalar_mul(
                                p[:], e[h][:, i * 256:(i + 1) * 256],
                                rec[h][:, i:i + 1])
                            nc.tensor.transpose(
                                ps_t[:, i * 256:i * 256 + 128],
                                p[:, 0:128], ident[:])
                            nc.tensor.transpose(
                                ps_t[:, i * 256 + 128:i * 256 + 256],
                                p[:, 128:256], ident[:])
                        nc.vector.tensor_copy(
                            pt_t[h][:, qp * 512 + 128:qp * 512 + 640],
                            ps_t[:])

                # PV for this pair
                y_even = psp.tile([128, TLOC], F32, tag="y", bufs=2,
                                  name=f"ye{g}")
                y_odd = psp.tile([128, TLOC], F32, tag="y", bufs=2,
                                 name=f"yo{g}")
                for jb in range(NKB):
                    i0 = max(0, (jb - 1) * 128)
                    i1 = min(TLOC, (jb + 1) * 128)
                    c0 = jb * 256 + (128 if jb == 0 else 0)
                    ps_y = y_even if jb % 2 == 0 else y_odd
                    for h in hs:
                        ho = (h % 2) * 64
                        nc.tensor.matmul(
                            ps_y[ho:ho + 64, i0:i1],
                            v_t[jb][:, h * 64:(h + 1) * 64],
                            pt_t[h][:, c0:c0 + (i1 - i0)],
                            start=True,
                            stop=True,
                            tile_position=(0, ho) if ho else None,
                        )
                yt = ytp.tile([128, TLOC], BF16, name=f"yt{g}", tag="yt",
                              bufs=KT)
                nc.scalar.copy(yt[:], y_even[:])
                nc.vector.tensor_tensor(out=yt[:], in0=yt[:], in1=y_odd[:],
                                        op=ADD)
                yt_t[g] = yt

            # ---- output projection z = y @ Wo^T: out (t, u)
            for tb in range(NQB):
                zt = zp.tile([128, C], F32, name=f"z{tb}", tag="z", bufs=2)
                for n0 in (0, 512):
                    ps = psp.tile([128, 512], F32, tag="proj", bufs=3, name=f"psz{tb}_{n0}")
                    for o in range(KT):
                        nc.tensor.matmul(
                            ps[:],
                            yt_t[o][:, tb * 128:(tb + 1) * 128],
                            wo_t[o][:, n0:n0 + 512],
                            start=(o == 0),
                            stop=(o == KT - 1),
                        )
                    nc.scalar.copy(zt[:, n0:n0 + 512], ps[:])
                nc.sync.dma_start(out_d[tb * 128:(tb + 1) * 128, :], zt[:])

    nc.compile()
    return nc


def _masks():
    il = np.arange(128)[:, None]
    jl = np.arange(256)[None, :]
    maskr = ((jl > il) & (jl <= il + 128))
    mask0 = (maskr & (jl >= 128))
    mbr = np.where(maskr, 0.0, -1e9).astype(np.float32)
    mb0 = np.where(mask0, 0.0, -1e9).astype(np.float32)
    return mb0, mbr


def make_in_maps(x, Wq, Wk, Wv, Wo):
    x = np.asarray(x, dtype=np.float32)
    xt = np.ascontiguousarray(x.reshape(T, C).T.astype(ml_dtypes.bfloat16))
    wqt = np.ascontiguousarray(
        (np.asarray(Wq, np.float32).T * np.float32(1.0 / np.sqrt(DH))
         ).astype(ml_dtypes.bfloat16))
    wkt = np.ascontiguousarray(np.asarray(Wk, np.float32).T.astype(ml_dtypes.bfloat16))
    wvt = np.ascontiguousarray(np.asarray(Wv, np.float32).T.astype(ml_dtypes.bfloat16))
    wot = np.ascontiguousarray(np.asarray(Wo, np.float32).T.astype(ml_dtypes.bfloat16))
    mask0, maskr = _masks()

    in_maps = []
    for c in range(NCORES):
        t0 = c * TLOC
        xs = np.zeros((C, KV), dtype=ml_dtypes.bfloat16)
        lo = t0 - HALO
        src_lo = max(lo, 0)
        xs[:, src_lo - lo:] = xt[:, src_lo:t0 + TLOC]
        in_maps.append(
            {
                "xt": xs,
                "wqt": wqt,
                "wkt": wkt,
                "wvt": wvt,
                "wot": wot,
                "mask0": mask0 if c == 0 else maskr,
                "maskr": maskr,
            }
        )
    return in_maps


def get_nc():
    if "nc" not in _cached:
        _cached["nc"] = build_nc()
    return _cached["nc"]


def kernel(x, Wq, Wk, Wv, Wo):
    in_maps = make_in_maps(x, Wq, Wk, Wv, Wo)
    res = run_bass_kernel_spmd(get_nc(), in_maps, list(range(NCORES)))
    out = np.concatenate([res.results[c]["out"] for c in range(NCORES)], axis=0)
    return out.reshape(1, T, C)


if __name__ == "__main__":
    rng = np.random.default_rng(0)
    ins = {
        "x": rng.standard_normal((1, T, C), dtype=np.float32),
        "Wq": rng.standard_normal((C, C), dtype=np.float32) * 0.02,
        "Wk": rng.standard_normal((C, C), dtype=np.float32) * 0.02,
        "Wv": rng.standard_normal((C, C), dtype=np.float32) * 0.02,
        "Wo": rng.standard_normal((C, C), dtype=np.float32) * 0.02,
    }
    out = kernel(**ins)
    print(out.shape, out.dtype, np.abs(out).mean())

